# revision 1
# baseline (speedup 1.0000x reference)
"""Trainium2 Bass kernel for nn_MultiHeadAttention_64647847739885.

Reference semantics (fp32):
    Wq_eff = softmax(Wq + tril_mask, axis=-2)   (if maskout else Wq)  [H,D,DK]
    Wk_eff = softmax(Wk + tril_mask, axis=-2)   (if maskout else Wk)
    WqQ = einsum('btd,hdk->bhtk', Q, Wq_eff)
    WkK = einsum('bsd,hdk->bhsk', K, Wk_eff)
    WvV = einsum('bsd,hdv->bhsv', V, Wv)
    scores = einsum('bhtk,bhsk->bhts', WqQ, WkK) / sqrt(dk)
    probs = softmax(scores, axis=-2)            # over the QUERY axis t!
    ctx = einsum('bhts,bhsv->bhtv', probs, WvV) -> (B,T,H*DV) @ Wo

Device strategy (8 NeuronCores, SPMD): core c handles batch b = c//2 and
head-group g = c%2 (heads_per_core=8, use_rs=True: pairwise ReduceScatter of
the partial output projection; each core emits its T/2 rows).  Fallback
variant: heads_per_core=16, use_rs=False (redundant pair, full output).
All activations live in transposed layouts so every matmul contracts over the
partition axis with natural tile loads; softmax over the query axis t becomes
a free-axis row softmax of scores^T; all softmax denominators fold into
per-partition scales.  Host does layout-only work (transpose/pack/slice) plus
constant mask/ones generation.
"""

import numpy as np

import concourse.bacc as bacc
import concourse.mybir as mybir
import concourse.tile as tile
from concourse import bass_utils
from concourse.bass_interp import get_hw_module

B, T, D = 4, 1024, 1024
H, DK = 16, 64
P = 128
N_CORES = 8
ND = D // P          # d tiles (contraction for projections)
NS = T // P          # s tiles
NT2 = T // 512       # moving-dim halves

F32 = mybir.dt.float32
BF16 = mybir.dt.bfloat16

RG_PAIRS = [[0, 1], [2, 3], [4, 5], [6, 7]]


def _emit_rep(nc, tc, aps, pp, tp, op_, psb, psc, maskout, HC, use_rs, rep,
              phases=frozenset({"loads", "weights", "wvv", "proj", "heads", "out"})):
    """Emit one full forward pass (or a phase subset, for timing only)."""
    NPAIR = HC // 2
    WCOLS = HC * DK
    NWC2 = max(1, WCOLS // 512)
    NMROW = WCOLS // P
    qT, kT, vT, wq, wk, wv, wo, tri, ones, out = aps

    qq = pp.tile([P, NPAIR, T], BF16, tag="qq")
    kk = pp.tile([P, NPAIR, T], BF16, tag="kk")
    wvv = pp.tile([P, NS, WCOLS], BF16, tag="wvv")
    ctx = pp.tile([P, NPAIR, T], BF16, tag="ctx")
    ones_t = pp.tile([P, 1], BF16, tag="ones")
    ones_f = pp.tile([P, 1], F32, tag="ones_f")
    qT_t = pp.tile([P, ND, T], BF16, tag="qT")
    kT_t = pp.tile([P, ND, T], BF16, tag="kT")
    vT_t = pp.tile([P, ND, T], BF16, tag="vT")
    wq_t = pp.tile([P, ND, WCOLS], BF16, tag="wq")
    wk_t = pp.tile([P, ND, WCOLS], BF16, tag="wk")
    wv_t = pp.tile([P, ND, WCOLS], BF16, tag="wv")
    wo_t = pp.tile([P, NMROW, D], BF16, tag="wo")
    tri_t = pp.tile([P, WCOLS], BF16, tag="tri")
    wqf = pp.tile([P, ND, WCOLS], F32, tag="wqf")
    wkf = pp.tile([P, ND, WCOLS], F32, tag="wkf")

    # ---- loads: HWDGE f32 into staging, DVE-cast to bf16 ----
    # (SWDGE cast-DMA measured ~137GB/s vs ~690GB/s for HWDGE f32)
    if "loads" in phases:
        nc.gpsimd.dma_start(ones_t[:], ones[:])
        nc.gpsimd.dma_start(ones_f[:], ones[:])
        if maskout:
            stt = tp.tile([P, WCOLS], F32, tag="stt")
            nc.sync.dma_start(stt[:], tri[:])
            nc.vector.tensor_copy(tri_t[:], stt[:])
        for j, (mat_ap, mat_t, nd_, w) in enumerate((
                (wq, wq_t, ND, WCOLS), (wk, wk_t, ND, WCOLS),
                (vT, vT_t, ND, T), (wv, wv_t, ND, WCOLS),
                (qT, qT_t, ND, T), (kT, kT_t, ND, T),
                (wo, wo_t, NMROW, D))):
            if maskout and j < 2:
                # f32 staging kept; the weight exp below does the bf16 cast
                dstf = wqf if j == 0 else wkf
                for i in range(nd_):
                    nc.sync.dma_start(dstf[:, i, :],
                                      mat_ap[i * P:(i + 1) * P, :])
                continue
            for i in range(nd_):
                st = tp.tile([P, T], F32, tag="st")
                nc.sync.dma_start(st[:, :w], mat_ap[i * P:(i + 1) * P, :])
                nc.vector.tensor_copy(mat_t[:, i, :], st[:, :w])

    # ---------------- weight softmax (exp in place + fold scales) --
    cscale = []  # per-pair (P,1) f32 scale folded into qq, or None
    if maskout and "weights" in phases:
        for w_f, w_t in ((wqf, wq_t), (wkf, wk_t)):
            for i in range(ND):
                nc.scalar.activation(
                    w_t[:, i, :], w_f[:, i, :],
                    mybir.ActivationFunctionType.Exp)
            # only d-tile 0 has masked entries (tril on (1024,64))
            nc.vector.tensor_mul(w_t[:, 0, :], w_t[:, 0, :], tri_t[:])
        # column sums over d via ones-stationary matmuls: (1 x WCOLS)
        sums_sb = []
        for w_t in (wq_t, wk_t):
            ps_s = psb.tile([P, 1024], F32, tag="big")
            for g in range(NWC2):
                gw = min(512, WCOLS)
                for i in range(ND):
                    nc.tensor.matmul(
                        ps_s[:1, g * 512:g * 512 + gw],
                        lhsT=ones_t[:],
                        rhs=w_t[:, i, g * 512:g * 512 + gw],
                        start=(i == 0), stop=(i == ND - 1))
            ssb = tp.tile([1, WCOLS], F32, tag="ssb")
            nc.vector.tensor_copy(ssb[:], ps_s[:1, :WCOLS])
            sums_sb.append(ssb)
        # transpose (1 x 128) slices into (128 x 1) via f32 matmul
        for p in range(NPAIR):
            ps_t = psb.tile([P, 1024], F32, tag="big")
            # two single-shot groups in different PSUM banks (cols 0 / 512)
            nc.tensor.matmul(
                ps_t[:, 0:1], lhsT=sums_sb[0][:, p * P:(p + 1) * P],
                rhs=ones_f[:1, :], start=True, stop=True)
            nc.tensor.matmul(
                ps_t[:, 512:513], lhsT=sums_sb[1][:, p * P:(p + 1) * P],
                rhs=ones_f[:1, :], start=True, stop=True)
            sqv = tp.tile([P, 1], F32, tag="sqv")
            nc.vector.tensor_copy(sqv[:], ps_t[:, 0:1])
            prod = tp.tile([P, 1], F32, tag="prod")
            nc.vector.tensor_mul(prod[:], sqv[:], ps_t[:, 512:513])
            c = tp.tile([P, 1], F32, tag=f"c{p}")
            nc.vector.reciprocal(c[:], prod[:])
            cscale.append(c)
    else:
        cscale = [None] * NPAIR

    # ---------------- wvv = (V @ Wv) in (s x v) --------------------
    for g in range(NWC2 if "wvv" in phases else 0):
        gw = min(512, WCOLS)
        for st in range(NS):
            ps = psb.tile([P, 1024], F32, tag="big")
            for i in range(ND):
                nc.tensor.matmul(
                    ps[:, :gw],
                    lhsT=vT_t[:, i, st * P:(st + 1) * P],
                    rhs=wv_t[:, i, g * 512:g * 512 + gw],
                    start=(i == 0), stop=(i == ND - 1),
                )
            nc.vector.tensor_copy(
                wvv[:, st, g * 512:g * 512 + gw], ps[:, :gw])

    # ---------------- per pair: projections then attention ---------
    for p in range(NPAIR if "proj" in phases else 0):
        ps = psb.tile([P, 1024], F32, tag="big")
        for i in range(ND):
            for n in range(NT2):
                nc.tensor.matmul(
                    ps[:, n * 512:(n + 1) * 512],
                    lhsT=wq_t[:, i, p * P:(p + 1) * P],
                    rhs=qT_t[:, i, n * 512:(n + 1) * 512],
                    start=(i == 0), stop=(i == ND - 1))
        if cscale[p] is not None:
            nc.vector.tensor_scalar_mul(qq[:, p, :], ps[:], cscale[p][:])
        else:
            nc.vector.tensor_copy(qq[:, p, :], ps[:])
        ps = psb.tile([P, 1024], F32, tag="big")
        for i in range(ND):
            for n in range(NT2):
                nc.tensor.matmul(
                    ps[:, n * 512:(n + 1) * 512],
                    lhsT=wk_t[:, i, p * P:(p + 1) * P],
                    rhs=kT_t[:, i, n * 512:(n + 1) * 512],
                    start=(i == 0), stop=(i == ND - 1))
        nc.vector.tensor_copy(kk[:, p, :], ps[:])

        # ---------------- attention for this pair ------------------
        if "heads" not in phases:
            continue
        pctx_a = psc.tile([P, T], F32, tag="ctxpA")
        pctx_b = psc.tile([P, T], F32, tag="ctxpB")
        pctx_h = (pctx_a, pctx_b)
        for st in range(NS):
            for half, base in ((0, 0), (1, 64)):
                pctx = pctx_h[half]
                psco = psb.tile([P, 1024], F32, tag="big")
                for n in range(NT2):
                    nc.tensor.matmul(
                        psco[:, n * 512:(n + 1) * 512],
                        lhsT=kk[base:base + 64, p, st * P:(st + 1) * P],
                        rhs=qq[base:base + 64, p, n * 512:(n + 1) * 512],
                        start=True, stop=True,
                        tile_position=(base, 0))
                e = tp.tile([P, T], BF16, tag="e")
                rs = tp.tile([P, 1], F32, tag="rs")
                nc.scalar.activation(
                    e[:], psco[:], mybir.ActivationFunctionType.Exp,
                    scale=0.125, accum_out=rs[:])
                r = tp.tile([P, 1], F32, tag="r")
                nc.vector.reciprocal(r[:], rs[:])
                hcol = (2 * p + half) * DK
                wvs = tp.tile([P, DK], BF16, tag="wvs")
                nc.vector.tensor_scalar_mul(
                    wvs[:], wvv[:, st, hcol:hcol + DK], r[:])
                for n in range(NT2):
                    nc.tensor.matmul(
                        pctx[base:base + 64, n * 512:(n + 1) * 512],
                        lhsT=wvs[:],
                        rhs=e[:, n * 512:(n + 1) * 512],
                        start=(st == 0), stop=(st == NS - 1),
                        tile_position=(0, base))
        nc.vector.tensor_copy(ctx[0:64, p, :], pctx_h[0][0:64, :])
        nc.vector.tensor_copy(ctx[64:128, p, :], pctx_h[1][64:128, :])

    # ---------------- Phase O: output projection -------------------
    if "out" not in phases:
        return
    if use_rs:
        dp_cm = tc.tile_pool(name=f"dram{rep}", bufs=1, space="DRAM")
        dp = dp_cm.__enter__()
        obounce = dp.tile([T, D], F32, tag="ob")
        ors1 = dp.tile([T // 4, D], F32, tag="ors1")
        ors2 = dp.tile([T // 4, D], F32, tag="ors2")
    for tt in range(T // P):
        pso = psb.tile([P, 1024], F32, tag="big")
        for m in range(NMROW):
            for n in range(NT2):
                nc.tensor.matmul(
                    pso[:, n * 512:(n + 1) * 512],
                    lhsT=ctx[:, m, tt * P:(tt + 1) * P],
                    rhs=wo_t[:, m, n * 512:(n + 1) * 512],
                    start=(m == 0), stop=(m == NMROW - 1))
        osb = op_.tile([P, D], F32, tag="o")
        nc.vector.tensor_copy(osb[:], pso[:])
        dst = obounce if use_rs else out
        nc.sync.dma_start(dst[tt * P:(tt + 1) * P, :], osb[:])
        if use_rs and tt == T // P // 2 - 1:
            # first-half RS overlaps the second half's output projection;
            # rank r receives rows [r*256, r*256+256) of each half-sum.
            nc.gpsimd.collective_compute(
                "ReduceScatter", mybir.AluOpType.add,
                replica_groups=RG_PAIRS,
                ins=[obounce[0:T // 2, :].opt()], outs=[ors1.opt()])
            nc.sync.dma_start(out[0:T // 4, :], ors1[:])
    if use_rs:
        nc.gpsimd.collective_compute(
            "ReduceScatter", mybir.AluOpType.add,
            replica_groups=RG_PAIRS,
            ins=[obounce[T // 2:T, :].opt()], outs=[ors2.opt()])
        nc.sync.dma_start(out[T // 4:T // 2, :], ors2[:])
        dp_cm.__exit__(None, None, None)


def _build(maskout: bool, heads_per_core: int, use_rs: bool, repeat: int = 1,
           loop_reps: int = 0,
           phases=frozenset({"loads", "weights", "wvv", "proj", "heads", "out"})):
    """Build + compile the SPMD program. Returns compiled nc.

    loop_reps > 0 wraps the body in a tc.For_i hardware loop (no collectives
    allowed in that mode) — used only for differential timing."""
    HC = heads_per_core
    WCOLS = HC * DK
    OUT_ROWS = T // 2 if use_rs else T

    nc = bacc.Bacc("TRN2", target_bir_lowering=False, debug=False,
                   num_devices=N_CORES)

    qT = nc.dram_tensor("qT", [D, T], F32, kind="ExternalInput").ap()
    kT = nc.dram_tensor("kT", [D, T], F32, kind="ExternalInput").ap()
    vT = nc.dram_tensor("vT", [D, T], F32, kind="ExternalInput").ap()
    wq = nc.dram_tensor("wq", [D, WCOLS], F32, kind="ExternalInput").ap()
    wk = nc.dram_tensor("wk", [D, WCOLS], F32, kind="ExternalInput").ap()
    wv = nc.dram_tensor("wv", [D, WCOLS], F32, kind="ExternalInput").ap()
    wo = nc.dram_tensor("wo", [WCOLS, D], F32, kind="ExternalInput").ap()
    tri = nc.dram_tensor("tri", [P, WCOLS], F32, kind="ExternalInput").ap()
    ones = nc.dram_tensor("ones", [P, 1], F32, kind="ExternalInput").ap()
    out = nc.dram_tensor("out", [OUT_ROWS, D], F32, kind="ExternalOutput").ap()
    aps = (qT, kT, vT, wq, wk, wv, wo, tri, ones, out)

    with tile.TileContext(nc) as tc:
        with (
            tc.tile_pool(name="persist", bufs=1) as pp,
            tc.tile_pool(name="trans", bufs=4) as tp,
            tc.tile_pool(name="osb", bufs=2) as op_,
            tc.tile_pool(name="psum_big", bufs=2, space="PSUM") as psb,
            tc.tile_pool(name="psum_ctx", bufs=1, space="PSUM") as psc,
        ):
            if loop_reps:
                assert not use_rs, "collectives cannot live inside For_i"
                with tc.For_i(0, loop_reps, 1):
                    _emit_rep(nc, tc, aps, pp, tp, op_, psb, psc,
                              maskout, HC, use_rs, 0, phases=phases)
            else:
                for rep in range(repeat):
                    _emit_rep(nc, tc, aps, pp, tp, op_, psb, psc,
                              maskout, HC, use_rs, rep, phases=phases)

    nc.compile()
    nc.m = get_hw_module(nc.m)
    return nc


_CACHE: dict = {}


def _get_program(maskout: bool, heads_per_core: int, use_rs: bool,
                 repeat: int = 1):
    key = (maskout, heads_per_core, use_rs, repeat)
    if key not in _CACHE:
        _CACHE[key] = _build(*key)
    return _CACHE[key]


def _prep_inputs(Q, K, V, Wq, Wk, Wv, Wo, heads_per_core):
    """Host-side layout-only sharding: per-core input dicts."""
    HC = heads_per_core
    WCOLS = HC * DK
    tri = (np.arange(P)[:, None] >= (np.arange(WCOLS)[None, :] % DK)) \
        .astype(np.float32)
    ones = np.ones((P, 1), np.float32)
    in_maps = []
    for c in range(N_CORES):
        b = c // 2
        if HC == H:
            hsel = np.arange(H)
        else:
            g = c % 2
            hsel = np.arange(g * HC, (g + 1) * HC)
        # (H,D,DK) -> (D, HC*DK) packed columns for selected heads
        wq_p = np.ascontiguousarray(
            Wq[hsel].transpose(1, 0, 2).reshape(D, WCOLS))
        wk_p = np.ascontiguousarray(
            Wk[hsel].transpose(1, 0, 2).reshape(D, WCOLS))
        wv_p = np.ascontiguousarray(
            Wv[hsel].transpose(1, 0, 2).reshape(D, WCOLS))
        wo_p = np.ascontiguousarray(Wo.reshape(H, DK, D)[hsel].reshape(WCOLS, D))
        in_maps.append({
            "qT": np.ascontiguousarray(Q[b].T),
            "kT": np.ascontiguousarray(K[b].T),
            "vT": np.ascontiguousarray(V[b].T),
            "wq": wq_p, "wk": wk_p, "wv": wv_p, "wo": wo_p,
            "tri": tri, "ones": ones,
        })
    return in_maps


def run(Q, K, V, Wq, Wk, Wv, Wo, maskout, heads_per_core=8, use_rs=True,
        repeat=1):
    Q = np.asarray(Q, np.float32)
    K = np.asarray(K, np.float32)
    V = np.asarray(V, np.float32)
    Wq = np.asarray(Wq, np.float32)
    Wk = np.asarray(Wk, np.float32)
    Wv = np.asarray(Wv, np.float32)
    Wo = np.asarray(Wo, np.float32)
    mk = bool(np.asarray(maskout).item())
    nc = _get_program(mk, heads_per_core, use_rs, repeat)
    in_maps = _prep_inputs(Q, K, V, Wq, Wk, Wv, Wo, heads_per_core)
    res = bass_utils.run_bass_kernel_spmd(
        nc, in_maps, list(range(N_CORES)), trace=False)
    outf = np.empty((B, T, D), np.float32)
    for c in range(N_CORES):
        b = c // 2
        if use_rs:
            r = c % 2
            o = res.results[c]["out"]  # rows: [sum half1 shard, sum half2 shard]
            outf[b, r * (T // 4):(r + 1) * (T // 4), :] = o[:T // 4]
            outf[b, T // 2 + r * (T // 4):T // 2 + (r + 1) * (T // 4), :] = \
                o[T // 4:]
        else:
            if c % 2 == 0:
                outf[b] = res.results[c]["out"]
    return outf, res


def kernel(Q, K, V, Wq, Wk, Wv, Wo, maskout):
    outf, _ = run(Q, K, V, Wq, Wk, Wv, Wo, maskout,
                  heads_per_core=8, use_rs=True)
    return outf



# revision 3
# speedup vs baseline: 1.2061x; 1.2061x over previous
"""Trainium2 Bass kernel for nn_MultiHeadAttention_64647847739885.

Reference semantics (fp32):
    Wq_eff = softmax(Wq + tril_mask, axis=-2)   (if maskout else Wq)  [H,D,DK]
    Wk_eff = softmax(Wk + tril_mask, axis=-2)
    WqQ = einsum('btd,hdk->bhtk', Q, Wq_eff)
    WkK = einsum('bsd,hdk->bhsk', K, Wk_eff)
    WvV = einsum('bsd,hdv->bhsv', V, Wv)
    scores = einsum('bhtk,bhsk->bhts', WqQ, WkK) / sqrt(dk)
    probs = softmax(scores, axis=-2)            # over the QUERY axis t!
    ctx = einsum('bhts,bhsv->bhtv', probs, WvV) -> (B,T,H*DV) @ Wo

Device strategy (8 NeuronCores, SPMD): core c handles batch b = c//2 and
head-group g = c%2 (8 heads = 4 head-pairs per core); pairwise ReduceScatter
of the partial output projection (each core emits its T/2 rows).

V2 over the original baseline:
  * host pre-casts all inputs to bf16 (halves DMA bytes, removes all
    f32->bf16 DVE cast copies on device)
  * software-pipelined attention: per (pair, s-tile) iteration the two
    heads' scores matmuls issue back-to-back on alternating PE row-groups
    (concurrent streams), exps queue on ScalarE immediately, and the ctx
    matmuls are deferred CTXLAG iterations so the ~73us/core ScalarE exp
    stream never waits on TensorE and vice versa
  * wvv chains and the projections for later pairs are pumped into the
    attention loop a few matmuls per iteration (PE slack absorbs them)
  * PSUM: 2x scores (4 banks) + chain pool (2) + ctx accum (2) = 8 banks
"""

import numpy as np
import ml_dtypes

import concourse.bacc as bacc
import concourse.mybir as mybir
import concourse.tile as tile
from concourse import bass_utils
from concourse.bass_interp import get_hw_module

B, T, D = 4, 1024, 1024
H, DK = 16, 64
P = 128
N_CORES = 8
HC = 8               # heads per core
NPAIR = HC // 2      # 4 head-pairs per core
WCOLS = HC * DK      # 512 packed weight columns per core
ND = D // P          # 8 contraction tiles for the projections
NS = T // P          # 8 s tiles
NT2 = T // 512       # 2 moving-dim halves
NMROW = WCOLS // P   # 4 ctx row-tiles for the output projection

CTXLAG = 2           # ctx matmuls trail scores by this many (p,st) iters
EBUFS = 2 * (CTXLAG + 1) + 2   # e/rs/r/wvs tile rotation depth

F32 = mybir.dt.float32
BF16 = mybir.dt.bfloat16
BF16NP = ml_dtypes.bfloat16

RG_PAIRS = [[0, 1], [2, 3], [4, 5], [6, 7]]


def _emit_rep(nc, tc, aps, pp, tp, op_, psb, psp, psc, maskout, use_rs, rep):
    """Emit one full forward pass."""
    qT, kT, vT, wq, wk, wv, wo, tri, ones, out = aps

    qq = pp.tile([P, NPAIR, T], BF16, tag="qq")
    kk = pp.tile([P, NPAIR, T], BF16, tag="kk")
    wvv = pp.tile([P, NS, WCOLS], BF16, tag="wvv")
    ctx = pp.tile([P, NPAIR, T], BF16, tag="ctx")
    ones_t = pp.tile([P, 1], BF16, tag="ones")
    ones_f = pp.tile([P, 1], F32, tag="ones_f")
    qT_t = pp.tile([P, ND, T], BF16, tag="qT")
    kT_t = pp.tile([P, ND, T], BF16, tag="kT")
    vT_t = pp.tile([P, ND, T], BF16, tag="vT")
    wq_t = pp.tile([P, ND, WCOLS], BF16, tag="wq")
    wk_t = pp.tile([P, ND, WCOLS], BF16, tag="wk")
    wv_t = pp.tile([P, ND, WCOLS], BF16, tag="wv")
    wo_t = pp.tile([P, NMROW, D], BF16, tag="wo")
    tri_t = pp.tile([P, WCOLS], BF16, tag="tri")

    # ---- loads: all bf16 (host pre-cast), straight into the SBUF tiles.
    # Priority order across the two HWDGE rings (sync / scalar): wq+wk gate
    # the weight exp, qT+kT gate pair-0/1 projections and thus the whole
    # exp stream; vT+wv gate only the (pipelined, lag-tolerant) ctx side;
    # wo is needed last.
    nc.gpsimd.dma_start(ones_t[:], ones[:])
    nc.gpsimd.dma_start(ones_f[:], ones[:])
    if maskout:
        nc.gpsimd.dma_start(tri_t[:], tri[:])
    for i in range(ND):
        nc.sync.dma_start(wq_t[:, i, :], wq[i * P:(i + 1) * P, :])
        nc.scalar.dma_start(wk_t[:, i, :], wk[i * P:(i + 1) * P, :])
    for i in range(ND):
        nc.sync.dma_start(qT_t[:, i, :], qT[i * P:(i + 1) * P, :])
        nc.scalar.dma_start(kT_t[:, i, :], kT[i * P:(i + 1) * P, :])
    for i in range(ND):
        nc.sync.dma_start(vT_t[:, i, :], vT[i * P:(i + 1) * P, :])
        nc.scalar.dma_start(wv_t[:, i, :], wv[i * P:(i + 1) * P, :])
    for m in range(NMROW):
        nc.sync.dma_start(wo_t[:, m, :], wo[m * P:(m + 1) * P, :])

    # ---------------- weight softmax (exp in place + fold scales) ------
    # cscale[p] (P,1 f32) = 1 / (colsum_q * colsum_k) per packed column,
    # folded into qq at evacuation time.
    cscale = [None] * NPAIR
    if maskout:
        nc.scalar.activation(wq_t[:, :, :], wq_t[:, :, :],
                             mybir.ActivationFunctionType.Exp)
        nc.scalar.activation(wk_t[:, :, :], wk_t[:, :, :],
                             mybir.ActivationFunctionType.Exp)
        # only d-tile 0 has masked entries (tril on (1024,64))
        nc.vector.tensor_mul(wq_t[:, 0, :], wq_t[:, 0, :], tri_t[:])
        nc.vector.tensor_mul(wk_t[:, 0, :], wk_t[:, 0, :], tri_t[:])
        # column sums over d via ones-stationary matmuls: (1 x WCOLS)
        sums_sb = []
        for w_t in (wq_t, wk_t):
            ps_s = psp.tile([P, 1024], F32, tag="pj")
            for i in range(ND):
                nc.tensor.matmul(
                    ps_s[:1, :WCOLS], lhsT=ones_t[:],
                    rhs=w_t[:, i, :],
                    start=(i == 0), stop=(i == ND - 1))
            ssb = tp.tile([1, WCOLS], F32, tag="ssb", bufs=2)
            nc.vector.tensor_copy(ssb[:], ps_s[:1, :WCOLS])
            sums_sb.append(ssb)
        # transpose (1 x 128) slices into (128 x 1) via f32 matmul
        for p in range(NPAIR):
            ps_t = psp.tile([P, 1024], F32, tag="pj")
            nc.tensor.matmul(
                ps_t[:, 0:1], lhsT=sums_sb[0][:, p * P:(p + 1) * P],
                rhs=ones_f[:1, :], start=True, stop=True)
            nc.tensor.matmul(
                ps_t[:, 512:513], lhsT=sums_sb[1][:, p * P:(p + 1) * P],
                rhs=ones_f[:1, :], start=True, stop=True)
            sqv = tp.tile([P, 1], F32, tag="sqv")
            nc.vector.tensor_copy(sqv[:], ps_t[:, 0:1])
            prod = tp.tile([P, 1], F32, tag="prod")
            nc.vector.tensor_mul(prod[:], sqv[:], ps_t[:, 512:513])
            c = tp.tile([P, 1], F32, tag=f"c{p}")
            nc.vector.reciprocal(c[:], prod[:])
            cscale[p] = c

    # ---------------- chain generators (one matmul per yield) ----------
    def gen_wvv(st):
        ps = psp.tile([P, 1024], F32, tag="pj")
        for i in range(ND):
            nc.tensor.matmul(
                ps[:, :WCOLS],
                lhsT=vT_t[:, i, st * P:(st + 1) * P],
                rhs=wv_t[:, i, :],
                start=(i == 0), stop=(i == ND - 1))
            if i == ND - 1:
                nc.vector.tensor_copy(wvv[:, st, :], ps[:, :WCOLS])
            yield

    def gen_proj(p, which):
        """which: 0 -> qq, 1 -> kk.  One PSUM chain (16 matmuls)."""
        w_t, dst = (wq_t, qq) if which == 0 else (wk_t, kk)
        src = qT_t if which == 0 else kT_t
        ps = psp.tile([P, 1024], F32, tag="pj")
        for i in range(ND):
            for n in range(NT2):
                nc.tensor.matmul(
                    ps[:, n * 512:(n + 1) * 512],
                    lhsT=w_t[:, i, p * P:(p + 1) * P],
                    rhs=src[:, i, n * 512:(n + 1) * 512],
                    start=(i == 0), stop=(i == ND - 1))
                if i == ND - 1 and n == NT2 - 1:
                    if which == 0 and cscale[p] is not None:
                        nc.vector.tensor_scalar_mul(
                            dst[:, p, :], ps[:], cscale[p][:])
                    else:
                        nc.vector.tensor_copy(dst[:, p, :], ps[:])
                yield

    def run_gen(g):
        for _ in g:
            pass

    # upfront: projections for pairs 0 and 1 (qq chains are gated on qT,
    # kk chains on kT; Tile starts each matmul as its d-tile lands).
    run_gen(gen_proj(0, 0))
    run_gen(gen_proj(1, 0))
    run_gen(gen_proj(0, 1))
    run_gen(gen_proj(1, 1))

    # filler stream pumped into the attention loop: all wvv chains (vT
    # arrives after kT, and ctx consumption is CTXLAG iters behind), then
    # projections for pairs 2 and 3.
    def filler_stream():
        for st in range(NS):
            yield from gen_wvv(st)
        for p in (2, 3):
            yield from gen_proj(p, 0)
            yield from gen_proj(p, 1)

    fill = filler_stream()
    # matmuls to pump per iteration index (32 iters):
    #   iters 0-7: one wvv chain each (wvv[st] emitted at iter st, just
    #   ahead of its first ctx consumer at iter st+CTXLAG)
    #   iters 8-15: proj2 (32 mms, due before iter 16)
    #   iters 16-23: proj3 (due before iter 24)
    pump = [8] * 8 + [4] * 8 + [4] * 8 + [0] * 8

    iters = [(p, st) for p in range(NPAIR) for st in range(NS)]
    NIT = len(iters)

    escale = 0.125  # 1/sqrt(DK)
    ework: list = [None] * NIT  # per-iter (e, r) handles for deferred ctx
    pctx = None
    ctx_pair = -1

    def emit_scores_exp(i):
        p, st = iters[i]
        ps_h = []
        for h in range(2):
            base = h * 64
            ps = psb.tile([P, 1024], F32, tag="sc")
            for n in range(NT2):
                nc.tensor.matmul(
                    ps[:, n * 512:(n + 1) * 512],
                    lhsT=kk[base:base + 64, p, st * P:(st + 1) * P],
                    rhs=qq[base:base + 64, p, n * 512:(n + 1) * 512],
                    start=True, stop=True,
                    tile_position=(base, 0))
            ps_h.append(ps)
        handles = []
        for h in range(2):
            e = tp.tile([P, T], BF16, tag="e", bufs=EBUFS)
            rs = tp.tile([P, 1], F32, tag="rs", bufs=EBUFS)
            nc.scalar.activation(
                e[:], ps_h[h][:], mybir.ActivationFunctionType.Exp,
                scale=escale, accum_out=rs[:])
            r = tp.tile([P, 1], F32, tag="r", bufs=EBUFS)
            nc.vector.reciprocal(r[:], rs[:])
            handles.append((e, r))
        ework[i] = handles

    def emit_ctx(i):
        nonlocal pctx, ctx_pair
        p, st = iters[i]
        if p != ctx_pair:
            if ctx_pair >= 0:
                nc.vector.tensor_copy(ctx[:, ctx_pair, :], pctx[:])
            pctx = psc.tile([P, T], F32, tag="ctx")
            ctx_pair = p
        for h in range(2):
            base = h * 64
            e, r = ework[i][h]
            hcol = (2 * p + h) * DK
            wvs = tp.tile([P, DK], BF16, tag="wvs", bufs=EBUFS)
            nc.vector.tensor_scalar_mul(
                wvs[:], wvv[:, st, hcol:hcol + DK], r[:])
            for n in range(NT2):
                nc.tensor.matmul(
                    pctx[base:base + 64, n * 512:(n + 1) * 512],
                    lhsT=wvs[:],
                    rhs=e[:, n * 512:(n + 1) * 512],
                    start=(st == 0), stop=(st == NS - 1),
                    tile_position=(0, base))
        ework[i] = None

    for i in range(NIT):
        emit_scores_exp(i)
        for _ in range(pump[i]):
            if next(fill, StopIteration) is StopIteration:
                break
        if i >= CTXLAG:
            emit_ctx(i - CTXLAG)
    for _ in fill:
        pass
    for i in range(NIT - CTXLAG, NIT):
        emit_ctx(i)
    nc.vector.tensor_copy(ctx[:, NPAIR - 1, :], pctx[:])

    # ---------------- Phase O: output projection -----------------------
    if use_rs:
        dp_cm = tc.tile_pool(name=f"dram{rep}", bufs=1, space="DRAM")
        dp = dp_cm.__enter__()
        obounce = dp.tile([T, D], F32, tag="ob")
        ors1 = dp.tile([T // 4, D], F32, tag="ors1")
        ors2 = dp.tile([T // 4, D], F32, tag="ors2")
    for tt in range(T // P):
        pso = psb.tile([P, 1024], F32, tag="sc")
        for m in range(NMROW):
            for n in range(NT2):
                nc.tensor.matmul(
                    pso[:, n * 512:(n + 1) * 512],
                    lhsT=ctx[:, m, tt * P:(tt + 1) * P],
                    rhs=wo_t[:, m, n * 512:(n + 1) * 512],
                    start=(m == 0), stop=(m == NMROW - 1))
        osb = op_.tile([P, D], F32, tag="o")
        nc.vector.tensor_copy(osb[:], pso[:])
        dst = obounce if use_rs else out
        nc.sync.dma_start(dst[tt * P:(tt + 1) * P, :], osb[:])
        if use_rs and tt == T // P // 2 - 1:
            # first-half RS overlaps the second half's output projection;
            # rank r receives rows [r*256, r*256+256) of each half-sum.
            nc.gpsimd.collective_compute(
                "ReduceScatter", mybir.AluOpType.add,
                replica_groups=RG_PAIRS,
                ins=[obounce[0:T // 2, :].opt()], outs=[ors1.opt()])
            nc.sync.dma_start(out[0:T // 4, :], ors1[:])
    if use_rs:
        nc.gpsimd.collective_compute(
            "ReduceScatter", mybir.AluOpType.add,
            replica_groups=RG_PAIRS,
            ins=[obounce[T // 2:T, :].opt()], outs=[ors2.opt()])
        nc.sync.dma_start(out[T // 4:T // 2, :], ors2[:])
        dp_cm.__exit__(None, None, None)


def _build(maskout: bool, use_rs: bool, repeat: int = 1, loop_reps: int = 0):
    """Build + compile the SPMD program. Returns compiled nc.

    loop_reps > 0 wraps the body in a tc.For_i hardware loop (no collectives
    allowed in that mode) -- used only for differential timing."""
    OUT_ROWS = T // 2 if use_rs else T

    nc = bacc.Bacc("TRN2", target_bir_lowering=False, debug=False,
                   num_devices=N_CORES)

    qT = nc.dram_tensor("qT", [D, T], BF16, kind="ExternalInput").ap()
    kT = nc.dram_tensor("kT", [D, T], BF16, kind="ExternalInput").ap()
    vT = nc.dram_tensor("vT", [D, T], BF16, kind="ExternalInput").ap()
    wq = nc.dram_tensor("wq", [D, WCOLS], BF16, kind="ExternalInput").ap()
    wk = nc.dram_tensor("wk", [D, WCOLS], BF16, kind="ExternalInput").ap()
    wv = nc.dram_tensor("wv", [D, WCOLS], BF16, kind="ExternalInput").ap()
    wo = nc.dram_tensor("wo", [WCOLS, D], BF16, kind="ExternalInput").ap()
    tri = nc.dram_tensor("tri", [P, WCOLS], BF16, kind="ExternalInput").ap()
    ones = nc.dram_tensor("ones", [P, 1], F32, kind="ExternalInput").ap()
    out = nc.dram_tensor("out", [OUT_ROWS, D], F32, kind="ExternalOutput").ap()
    aps = (qT, kT, vT, wq, wk, wv, wo, tri, ones, out)

    with tile.TileContext(nc) as tc:
        with (
            tc.tile_pool(name="persist", bufs=1) as pp,
            tc.tile_pool(name="trans", bufs=4) as tp,
            tc.tile_pool(name="osb", bufs=2) as op_,
            tc.tile_pool(name="psum_sc", bufs=2, space="PSUM") as psb,
            tc.tile_pool(name="psum_pj", bufs=1, space="PSUM") as psp,
            tc.tile_pool(name="psum_ctx", bufs=1, space="PSUM") as psc,
        ):
            if loop_reps:
                assert not use_rs, "collectives cannot live inside For_i"
                with tc.For_i(0, loop_reps, 1):
                    _emit_rep(nc, tc, aps, pp, tp, op_, psb, psp, psc,
                              maskout, use_rs, 0)
            else:
                for rep in range(repeat):
                    _emit_rep(nc, tc, aps, pp, tp, op_, psb, psp, psc,
                              maskout, use_rs, rep)

    nc.compile()
    nc.m = get_hw_module(nc.m)
    return nc


_CACHE: dict = {}


def _get_program(maskout: bool, use_rs: bool, repeat: int = 1):
    key = (maskout, use_rs, repeat)
    if key not in _CACHE:
        _CACHE[key] = _build(*key)
    return _CACHE[key]


def _prep_inputs(Q, K, V, Wq, Wk, Wv, Wo, heads_per_core=HC):
    """Host-side sharding: per-core input dicts (bf16 pre-cast + layout)."""
    tri = (np.arange(P)[:, None] >= (np.arange(WCOLS)[None, :] % DK)) \
        .astype(BF16NP)
    ones = np.ones((P, 1), np.float32)
    in_maps = []
    for c in range(N_CORES):
        b = c // 2
        g = c % 2
        hsel = np.arange(g * HC, (g + 1) * HC)
        # (H,D,DK) -> (D, HC*DK) packed columns for selected heads
        wq_p = np.ascontiguousarray(
            Wq[hsel].transpose(1, 0, 2).reshape(D, WCOLS)).astype(BF16NP)
        wk_p = np.ascontiguousarray(
            Wk[hsel].transpose(1, 0, 2).reshape(D, WCOLS)).astype(BF16NP)
        wv_p = np.ascontiguousarray(
            Wv[hsel].transpose(1, 0, 2).reshape(D, WCOLS)).astype(BF16NP)
        wo_p = np.ascontiguousarray(
            Wo.reshape(H, DK, D)[hsel].reshape(WCOLS, D)).astype(BF16NP)
        in_maps.append({
            "qT": np.ascontiguousarray(Q[b].T).astype(BF16NP),
            "kT": np.ascontiguousarray(K[b].T).astype(BF16NP),
            "vT": np.ascontiguousarray(V[b].T).astype(BF16NP),
            "wq": wq_p, "wk": wk_p, "wv": wv_p, "wo": wo_p,
            "tri": tri, "ones": ones,
        })
    return in_maps


def run(Q, K, V, Wq, Wk, Wv, Wo, maskout, use_rs=True, repeat=1):
    Q = np.asarray(Q, np.float32)
    K = np.asarray(K, np.float32)
    V = np.asarray(V, np.float32)
    Wq = np.asarray(Wq, np.float32)
    Wk = np.asarray(Wk, np.float32)
    Wv = np.asarray(Wv, np.float32)
    Wo = np.asarray(Wo, np.float32)
    mk = bool(np.asarray(maskout).item())
    nc = _get_program(mk, use_rs, repeat)
    in_maps = _prep_inputs(Q, K, V, Wq, Wk, Wv, Wo)
    res = bass_utils.run_bass_kernel_spmd(
        nc, in_maps, list(range(N_CORES)), trace=False)
    outf = np.empty((B, T, D), np.float32)
    for c in range(N_CORES):
        b = c // 2
        if use_rs:
            r = c % 2
            o = res.results[c]["out"]  # rows: [sum half1 shard, sum half2 shard]
            outf[b, r * (T // 4):(r + 1) * (T // 4), :] = o[:T // 4]
            outf[b, T // 2 + r * (T // 4):T // 2 + (r + 1) * (T // 4), :] = \
                o[T // 4:]
        else:
            if c % 2 == 0:
                outf[b] = res.results[c]["out"]
    return outf, res


def kernel(Q, K, V, Wq, Wk, Wv, Wo, maskout):
    outf, _ = run(Q, K, V, Wq, Wk, Wv, Wo, maskout, use_rs=True)
    return outf


# revision 13
# speedup vs baseline: 1.7272x; 1.4321x over previous
"""Trainium2 Bass kernel for nn_MultiHeadAttention_64647847739885.

Reference semantics (fp32):
    Wq_eff = softmax(Wq + tril_mask, axis=-2)   (if maskout else Wq)  [H,D,DK]
    Wk_eff = softmax(Wk + tril_mask, axis=-2)
    WqQ = einsum('btd,hdk->bhtk', Q, Wq_eff)
    WkK = einsum('bsd,hdk->bhsk', K, Wk_eff)
    WvV = einsum('bsd,hdv->bhsv', V, Wv)
    scores = einsum('bhtk,bhsk->bhts', WqQ, WkK) / sqrt(dk)
    probs = softmax(scores, axis=-2)            # over the QUERY axis t!
    ctx = einsum('bhts,bhsv->bhtv', probs, WvV) -> (B,T,H*DV) @ Wo

Device strategy (8 NeuronCores, SPMD): core c handles batch b = c//2 and
head-group g = c%2 (8 heads = 4 head-pairs per core); pairwise ReduceScatter
of the partial output projection (each core emits its T/2 rows).

V2 over the original baseline:
  * host pre-casts all inputs to bf16 (halves DMA bytes, removes all
    f32->bf16 DVE cast copies on device)
  * software-pipelined attention: per (pair, s-tile) iteration the two
    heads' scores matmuls issue back-to-back on alternating PE row-groups
    (concurrent streams), exps queue on ScalarE immediately, and the ctx
    matmuls are deferred CTXLAG iterations so the ~73us/core ScalarE exp
    stream never waits on TensorE and vice versa
  * wvv chains and the projections for later pairs are pumped into the
    attention loop a few matmuls per iteration (PE slack absorbs them)
  * PSUM: 2x scores (4 banks) + chain pool (2) + ctx accum (2) = 8 banks
"""

import numpy as np
import ml_dtypes

import concourse.bacc as bacc
import concourse.mybir as mybir
import concourse.tile as tile
from concourse import bass_utils
from concourse.bass_interp import get_hw_module

B, T, D = 4, 1024, 1024
H, DK = 16, 64
P = 128
N_CORES = 8
HC = 8               # heads per core
NPAIR = HC // 2      # 4 head-pairs per core
WCOLS = HC * DK      # 512 packed weight columns per core
ND = D // P          # 8 contraction tiles for the projections
NS = T // P          # 8 s tiles
NT2 = T // 512       # 2 moving-dim halves
NMROW = WCOLS // P   # 4 ctx row-tiles for the output projection

CTXLAG = 6           # ctx matmuls trail scores by this many (p,st) iters
EBUFS = 2 * (CTXLAG + 1) + 2   # e/rs/r/wvs tile rotation depth

F32 = mybir.dt.float32
BF16 = mybir.dt.bfloat16
BF16NP = ml_dtypes.bfloat16

RG_PAIRS = [[0, 1], [2, 3], [4, 5], [6, 7]]


def _emit_rep(nc, tc, aps, pp, tp, op_, psb, psp, psc, maskout, use_rs, rep):
    """Emit one full forward pass."""
    qT, kT, vT, wq, wk, wv, wo, tri, ones, out = aps

    qq = pp.tile([P, NPAIR, T], BF16, tag="qq")
    kk = pp.tile([P, NPAIR, T], BF16, tag="kk")
    wvv = pp.tile([P, NS, WCOLS], BF16, tag="wvv")
    ctx = pp.tile([P, NPAIR, T], BF16, tag="ctx")
    ones_t = pp.tile([P, 1], BF16, tag="ones")
    ones_f = pp.tile([P, 1], F32, tag="ones_f")
    qT_t = pp.tile([P, ND, T], BF16, tag="qT")
    kT_t = pp.tile([P, ND, T], BF16, tag="kT")
    vT_t = pp.tile([P, ND, T], BF16, tag="vT")
    wq_t = pp.tile([P, ND, WCOLS], BF16, tag="wq")
    wk_t = pp.tile([P, ND, WCOLS], BF16, tag="wk")
    wv_t = pp.tile([P, ND, WCOLS], BF16, tag="wv")
    wo_t = pp.tile([P, NMROW, D], BF16, tag="wo")
    tri_t = pp.tile([P, WCOLS], BF16, tag="tri")

    # ---- loads: all bf16 (host pre-cast), straight into the SBUF tiles.
    # Everything goes on the SP (sync) HWDGE ring -- scalar-ring DMA
    # triggers would occupy the ACT sequencer (~667ns each) and delay the
    # exp stream.  Whole-tensor loads use a rearranged DRAM view (1 trigger);
    # qT/kT/vT stay per-d-tile so the projection chains can start as tiles
    # land.  Priority: wq+wk gate the weight exp, qT+kT gate the pair-0
    # projections and thus the whole exp stream, vT+wv gate only the
    # (lag-tolerant) ctx side, wo is needed last.
    nc.gpsimd.dma_start(ones_t[:], ones[:])
    nc.gpsimd.dma_start(ones_f[:], ones[:])
    if maskout:
        nc.gpsimd.dma_start(tri_t[:], tri[:])
    # ~10 big transfers: each HWDGE trigger costs ~565ns of SP sequencer
    # time, so per-d-tile DMAs would be issue-rate-bound; per-half-tensor
    # transfers keep the DMA engines saturated while still letting the
    # projection chains start on the first half.
    HALF = ND // 2
    nc.sync.dma_start(wq_t[:, :, :], wq.rearrange("(i p) c -> p i c", p=P))
    nc.sync.dma_start(wk_t[:, :, :], wk.rearrange("(i p) c -> p i c", p=P))
    for hh in range(2):
        r = slice(hh * HALF * P, (hh + 1) * HALF * P)
        nc.sync.dma_start(qT_t[:, hh * HALF:(hh + 1) * HALF, :],
                          qT[r, :].rearrange("(i p) t -> p i t", p=P))
    for hh in range(2):
        r = slice(hh * HALF * P, (hh + 1) * HALF * P)
        nc.sync.dma_start(kT_t[:, hh * HALF:(hh + 1) * HALF, :],
                          kT[r, :].rearrange("(i p) t -> p i t", p=P))
    nc.sync.dma_start(wv_t[:, :, :], wv.rearrange("(i p) c -> p i c", p=P))
    for hh in range(2):
        r = slice(hh * HALF * P, (hh + 1) * HALF * P)
        nc.sync.dma_start(vT_t[:, hh * HALF:(hh + 1) * HALF, :],
                          vT[r, :].rearrange("(i p) t -> p i t", p=P))
    nc.sync.dma_start(wo_t[:, :, :], wo.rearrange("(m p) c -> p m c", p=P))

    # ---------------- weight softmax (exp in place + fold scales) ------
    # cscale[p] (P,1 f32) = 1 / (colsum_q * colsum_k) per packed column,
    # folded into kk at evacuation time (kk chains finish after the sums,
    # so the fold costs nothing; folding into qq would deadlock the psp
    # chain pool: qq evac would wait on sums that need the pool).
    cscale = [None] * NPAIR
    if maskout:
        # per-d-tile exps so each projection-chain matmul only waits on its
        # own tile's exp (overlaps the qT/kT DMA stream)
        for i in range(ND):
            nc.scalar.activation(wq_t[:, i, :], wq_t[:, i, :],
                                 mybir.ActivationFunctionType.Exp)
        for i in range(ND):
            nc.scalar.activation(wk_t[:, i, :], wk_t[:, i, :],
                                 mybir.ActivationFunctionType.Exp)
        # only d-tile 0 has masked entries (tril on (1024,64))
        nc.vector.tensor_mul(wq_t[:, 0, :], wq_t[:, 0, :], tri_t[:])
        nc.vector.tensor_mul(wk_t[:, 0, :], wk_t[:, 0, :], tri_t[:])

    def emit_sums():
        # column sums over d via ones-stationary matmuls: (1 x WCOLS)
        sums_sb = []
        for w_t in (wq_t, wk_t):
            ps_s = psp.tile([P, 1024], F32, tag="pj")
            for i in range(ND):
                nc.tensor.matmul(
                    ps_s[:1, :WCOLS], lhsT=ones_t[:],
                    rhs=w_t[:, i, :],
                    start=(i == 0), stop=(i == ND - 1))
            ssb = tp.tile([1, WCOLS], F32, tag="ssb", bufs=2)
            nc.vector.tensor_copy(ssb[:], ps_s[:1, :WCOLS])
            sums_sb.append(ssb)
        # transpose (1 x 128) slices into (128 x 1) via f32 matmul
        for p in range(NPAIR):
            ps_t = psp.tile([P, 1024], F32, tag="pj")
            nc.tensor.matmul(
                ps_t[:, 0:1], lhsT=sums_sb[0][:, p * P:(p + 1) * P],
                rhs=ones_f[:1, :], start=True, stop=True)
            nc.tensor.matmul(
                ps_t[:, 512:513], lhsT=sums_sb[1][:, p * P:(p + 1) * P],
                rhs=ones_f[:1, :], start=True, stop=True)
            sqv = tp.tile([P, 1], F32, tag="sqv")
            nc.vector.tensor_copy(sqv[:], ps_t[:, 0:1])
            prod = tp.tile([P, 1], F32, tag="prod")
            nc.vector.tensor_mul(prod[:], sqv[:], ps_t[:, 512:513])
            c = tp.tile([P, 1], F32, tag=f"c{p}")
            nc.vector.reciprocal(c[:], prod[:])
            cscale[p] = c

    # ---------------- chain generators (one matmul per yield) ----------
    def gen_wvv(st):
        ps = psp.tile([P, 1024], F32, tag="pj")
        for i in range(ND):
            nc.tensor.matmul(
                ps[:, :WCOLS],
                lhsT=vT_t[:, i, st * P:(st + 1) * P],
                rhs=wv_t[:, i, :],
                start=(i == 0), stop=(i == ND - 1))
            if i == ND - 1:
                nc.vector.tensor_copy(wvv[:, st, :], ps[:, :WCOLS])
            yield

    def gen_proj(p, which):
        """which: 0 -> qq, 1 -> kk.  One PSUM chain (16 matmuls)."""
        w_t, dst = (wq_t, qq) if which == 0 else (wk_t, kk)
        src = qT_t if which == 0 else kT_t
        ps = psp.tile([P, 1024], F32, tag="pj")
        for i in range(ND):
            for n in range(NT2):
                nc.tensor.matmul(
                    ps[:, n * 512:(n + 1) * 512],
                    lhsT=w_t[:, i, p * P:(p + 1) * P],
                    rhs=src[:, i, n * 512:(n + 1) * 512],
                    start=(i == 0), stop=(i == ND - 1))
                if i == ND - 1 and n == NT2 - 1:
                    if which == 1 and cscale[p] is not None:
                        nc.vector.tensor_scalar_mul(
                            dst[:, p, :], ps[:], cscale[p][:])
                    else:
                        nc.vector.tensor_copy(dst[:, p, :], ps[:])
                yield

    def run_gen(g):
        for _ in g:
            pass

    # upfront: pair-0 projections around the softmax sums.  PE order is
    # proj0qq (gated on exp_wq + qT) -> sums/cscale -> proj0kk (gated on
    # exp_wk + kT; its evac folds cscale) -> first scores.
    run_gen(gen_proj(0, 0))
    if maskout:
        emit_sums()
    run_gen(gen_proj(0, 1))

    # filler stream pumped into the attention loop.  proj1 is due before
    # iter 8; wvv[st] is due before iter st+CTXLAG; proj2/proj3 before
    # iters 16/24.  At a uniform 8 matmuls/iter every deadline is met.
    def filler_stream():
        yield from gen_proj(1, 0)
        yield from gen_proj(1, 1)
        for st in range(NS):
            yield from gen_wvv(st)
        for p in (2, 3):
            yield from gen_proj(p, 0)
            yield from gen_proj(p, 1)

    fill = filler_stream()
    pump = [8] * 20 + [0] * 12

    iters = [(p, st) for p in range(NPAIR) for st in range(NS)]
    NIT = len(iters)

    escale = 0.125  # 1/sqrt(DK)
    ework: list = [None] * NIT  # per-iter (e, r) handles for deferred ctx
    pctx = None
    ctx_pair = -1

    def emit_scores_exp(i):
        p, st = iters[i]
        ps_h = []
        for h in range(2):
            base = h * 64
            ps = psb.tile([P, 1024], F32, tag="sc")
            for n in range(NT2):
                nc.tensor.matmul(
                    ps[:, n * 512:(n + 1) * 512],
                    lhsT=kk[base:base + 64, p, st * P:(st + 1) * P],
                    rhs=qq[base:base + 64, p, n * 512:(n + 1) * 512],
                    start=True, stop=True,
                    tile_position=(base, 0))
            ps_h.append(ps)
        handles = []
        for h in range(2):
            e = tp.tile([P, T], BF16, tag="e", bufs=EBUFS)
            rs = tp.tile([P, 1], F32, tag="rs", bufs=EBUFS)
            nc.scalar.activation(
                e[:], ps_h[h][:], mybir.ActivationFunctionType.Exp,
                scale=escale, accum_out=rs[:])
            r = tp.tile([P, 1], F32, tag="r", bufs=EBUFS)
            nc.vector.reciprocal(r[:], rs[:])
            handles.append((e, r))
        ework[i] = handles

    def emit_ctx(i):
        nonlocal pctx, ctx_pair
        p, st = iters[i]
        if p != ctx_pair:
            if ctx_pair >= 0:
                nc.vector.tensor_copy(ctx[:, ctx_pair, :], pctx[:])
            pctx = psc.tile([P, T], F32, tag="ctx")
            ctx_pair = p
        for h in range(2):
            base = h * 64
            e, r = ework[i][h]
            hcol = (2 * p + h) * DK
            wvs = tp.tile([P, DK], BF16, tag="wvs", bufs=EBUFS)
            nc.vector.tensor_scalar_mul(
                wvs[:], wvv[:, st, hcol:hcol + DK], r[:])
            for n in range(NT2):
                nc.tensor.matmul(
                    pctx[base:base + 64, n * 512:(n + 1) * 512],
                    lhsT=wvs[:],
                    rhs=e[:, n * 512:(n + 1) * 512],
                    start=(st == 0), stop=(st == NS - 1),
                    tile_position=(0, base))
        ework[i] = None

    # pump after scores: at 8/iter every chain still completes before its
    # first consumer (proj2 finishes in iter 15's pump, sc(2,0) is iter 16;
    # wvv[st] finishes by iter st+4, ctx(0,st) is iter st+CTXLAG).
    for i in range(NIT):
        emit_scores_exp(i)
        for _ in range(pump[i]):
            if next(fill, StopIteration) is StopIteration:
                break
        if i >= CTXLAG:
            emit_ctx(i - CTXLAG)
    for _ in fill:
        pass
    for i in range(NIT - CTXLAG, NIT):
        emit_ctx(i)
    nc.vector.tensor_copy(ctx[:, NPAIR - 1, :], pctx[:])

    # ---------------- Phase O: output projection -----------------------
    # bf16 partials/output (host upcasts): halves the out-DMA and the
    # ReduceScatter traffic.
    if use_rs:
        dp_cm = tc.tile_pool(name=f"dram{rep}", bufs=1, space="DRAM")
        dp = dp_cm.__enter__()
        obounce = dp.tile([T, D], BF16, tag="ob")
        ors1 = dp.tile([T // 4, D], BF16, tag="ors1")
        ors2 = dp.tile([T // 4, D], BF16, tag="ors2")
    for tt in range(T // P):
        pso = psb.tile([P, 1024], F32, tag="sc")
        for m in range(NMROW):
            for n in range(NT2):
                nc.tensor.matmul(
                    pso[:, n * 512:(n + 1) * 512],
                    lhsT=ctx[:, m, tt * P:(tt + 1) * P],
                    rhs=wo_t[:, m, n * 512:(n + 1) * 512],
                    start=(m == 0), stop=(m == NMROW - 1))
        osb = op_.tile([P, D], BF16, tag="o", bufs=3)
        nc.vector.tensor_copy(osb[:], pso[:])
        dst = obounce if use_rs else out
        nc.sync.dma_start(dst[tt * P:(tt + 1) * P, :], osb[:])
        if use_rs and tt == T // P // 2 - 1:
            # first-half RS overlaps the second half's output projection;
            # rank r receives rows [r*256, r*256+256) of each half-sum.
            nc.gpsimd.collective_compute(
                "ReduceScatter", mybir.AluOpType.add,
                replica_groups=RG_PAIRS,
                ins=[obounce[0:T // 2, :].opt()], outs=[ors1.opt()])
            nc.sync.dma_start(out[0:T // 4, :], ors1[:])
    if use_rs:
        nc.gpsimd.collective_compute(
            "ReduceScatter", mybir.AluOpType.add,
            replica_groups=RG_PAIRS,
            ins=[obounce[T // 2:T, :].opt()], outs=[ors2.opt()])
        nc.sync.dma_start(out[T // 4:T // 2, :], ors2[:])
        dp_cm.__exit__(None, None, None)


def _build(maskout: bool, use_rs: bool, repeat: int = 1, loop_reps: int = 0):
    """Build + compile the SPMD program. Returns compiled nc.

    loop_reps > 0 wraps the body in a tc.For_i hardware loop (no collectives
    allowed in that mode) -- used only for differential timing."""
    OUT_ROWS = T // 2 if use_rs else T

    nc = bacc.Bacc("TRN2", target_bir_lowering=False, debug=False,
                   num_devices=N_CORES)

    qT = nc.dram_tensor("qT", [D, T], BF16, kind="ExternalInput").ap()
    kT = nc.dram_tensor("kT", [D, T], BF16, kind="ExternalInput").ap()
    vT = nc.dram_tensor("vT", [D, T], BF16, kind="ExternalInput").ap()
    wq = nc.dram_tensor("wq", [D, WCOLS], BF16, kind="ExternalInput").ap()
    wk = nc.dram_tensor("wk", [D, WCOLS], BF16, kind="ExternalInput").ap()
    wv = nc.dram_tensor("wv", [D, WCOLS], BF16, kind="ExternalInput").ap()
    wo = nc.dram_tensor("wo", [WCOLS, D], BF16, kind="ExternalInput").ap()
    tri = nc.dram_tensor("tri", [P, WCOLS], BF16, kind="ExternalInput").ap()
    ones = nc.dram_tensor("ones", [P, 1], F32, kind="ExternalInput").ap()
    out = nc.dram_tensor("out", [OUT_ROWS, D], BF16, kind="ExternalOutput").ap()
    aps = (qT, kT, vT, wq, wk, wv, wo, tri, ones, out)

    with tile.TileContext(nc) as tc:
        with (
            tc.tile_pool(name="persist", bufs=1) as pp,
            tc.tile_pool(name="trans", bufs=4) as tp,
            tc.tile_pool(name="osb", bufs=2) as op_,
            tc.tile_pool(name="psum_sc", bufs=2, space="PSUM") as psb,
            tc.tile_pool(name="psum_pj", bufs=1, space="PSUM") as psp,
            tc.tile_pool(name="psum_ctx", bufs=1, space="PSUM") as psc,
        ):
            if loop_reps:
                assert not use_rs, "collectives cannot live inside For_i"
                with tc.For_i(0, loop_reps, 1):
                    _emit_rep(nc, tc, aps, pp, tp, op_, psb, psp, psc,
                              maskout, use_rs, 0)
            else:
                for rep in range(repeat):
                    _emit_rep(nc, tc, aps, pp, tp, op_, psb, psp, psc,
                              maskout, use_rs, rep)

    nc.compile()
    nc.m = get_hw_module(nc.m)
    return nc


_CACHE: dict = {}


def _get_program(maskout: bool, use_rs: bool, repeat: int = 1):
    key = (maskout, use_rs, repeat)
    if key not in _CACHE:
        _CACHE[key] = _build(*key)
    return _CACHE[key]


def _prep_inputs(Q, K, V, Wq, Wk, Wv, Wo, heads_per_core=HC):
    """Host-side sharding: per-core input dicts (bf16 pre-cast + layout)."""
    tri = (np.arange(P)[:, None] >= (np.arange(WCOLS)[None, :] % DK)) \
        .astype(BF16NP)
    ones = np.ones((P, 1), np.float32)
    in_maps = []
    for c in range(N_CORES):
        b = c // 2
        g = c % 2
        hsel = np.arange(g * HC, (g + 1) * HC)
        # (H,D,DK) -> (D, HC*DK) packed columns for selected heads
        wq_p = np.ascontiguousarray(
            Wq[hsel].transpose(1, 0, 2).reshape(D, WCOLS)).astype(BF16NP)
        wk_p = np.ascontiguousarray(
            Wk[hsel].transpose(1, 0, 2).reshape(D, WCOLS)).astype(BF16NP)
        wv_p = np.ascontiguousarray(
            Wv[hsel].transpose(1, 0, 2).reshape(D, WCOLS)).astype(BF16NP)
        wo_p = np.ascontiguousarray(
            Wo.reshape(H, DK, D)[hsel].reshape(WCOLS, D)).astype(BF16NP)
        in_maps.append({
            "qT": np.ascontiguousarray(Q[b].T).astype(BF16NP),
            "kT": np.ascontiguousarray(K[b].T).astype(BF16NP),
            "vT": np.ascontiguousarray(V[b].T).astype(BF16NP),
            "wq": wq_p, "wk": wk_p, "wv": wv_p, "wo": wo_p,
            "tri": tri, "ones": ones,
        })
    return in_maps


def run(Q, K, V, Wq, Wk, Wv, Wo, maskout, use_rs=True, repeat=1):
    Q = np.asarray(Q, np.float32)
    K = np.asarray(K, np.float32)
    V = np.asarray(V, np.float32)
    Wq = np.asarray(Wq, np.float32)
    Wk = np.asarray(Wk, np.float32)
    Wv = np.asarray(Wv, np.float32)
    Wo = np.asarray(Wo, np.float32)
    mk = bool(np.asarray(maskout).item())
    nc = _get_program(mk, use_rs, repeat)
    in_maps = _prep_inputs(Q, K, V, Wq, Wk, Wv, Wo)
    res = bass_utils.run_bass_kernel_spmd(
        nc, in_maps, list(range(N_CORES)), trace=False)
    outf = np.empty((B, T, D), np.float32)
    for c in range(N_CORES):
        b = c // 2
        if use_rs:
            r = c % 2
            o = np.asarray(res.results[c]["out"], np.float32)
            outf[b, r * (T // 4):(r + 1) * (T // 4), :] = o[:T // 4]
            outf[b, T // 2 + r * (T // 4):T // 2 + (r + 1) * (T // 4), :] = \
                o[T // 4:]
        else:
            if c % 2 == 0:
                outf[b] = np.asarray(res.results[c]["out"], np.float32)
    return outf, res


def kernel(Q, K, V, Wq, Wk, Wv, Wo, maskout):
    outf, _ = run(Q, K, V, Wq, Wk, Wv, Wo, maskout, use_rs=True)
    return outf
